# revision 40
# baseline (speedup 1.0000x reference)
"""Sparse multi-head attention (nn_MultiHeadAttention_44332652429419) on 8 trn2 cores.

Strategy v2 (tensor-parallel over H=16 heads, 2 heads per core, all-bf16 PE):
  Host: compose the stacked linears into one weight per tensor (bf16);
        drop bk (cancels in softmax) and bv (folds into the output bias,
        since attention rows sum to 1); build the multiplicative mask
        exp(additive) transposed, f16; transpose x (bf16).
  Device (per core, SPMD with per-core weight slices):
    q2T/k2T/v2T = W_c @ x.T (+ bq via ones-row)   [128, 3072] bf16
    per (strip of 512 queries, key tile mk):
      scoresT[h] = k2z_h.T @ q2s  -> paired PSUM [128, 2, 512]
      p = exp(scores * 1/8)       one ACT op over both heads (1024 free)
      pm = p * maskT tile         one DVE op, 16-bit (mask broadcast over h)
      ps_o[h] += vaug_h @ pm_h    PV accumulation over mk (ones-col rowsum)
    epilogue per strip: normalize by rowsum, out_proj partial -> ypart bf16
  Host: y = sum_c ypart_c + (bv_eff @ wo.T + bo)
"""
import os
import sys

sys.path.insert(0, "/opt/trn_rl_repo")

import numpy as np
import ml_dtypes
from contextlib import ExitStack

import concourse.bass as bass
import concourse.bacc as bacc
import concourse.mybir as mybir
import concourse.tile as tile
from concourse.bass_utils import run_bass_kernel_spmd
from concourse.masks import make_identity

F32 = mybir.dt.float32
F32R = mybir.dt.float32r
BF16 = mybir.dt.bfloat16
F16 = mybir.dt.float16
AF = mybir.ActivationFunctionType
ALU = mybir.AluOpType

N = 3072
IN_F = 1024
OUT_F = 1024
H = 16
D = 64
NCORES = 8
HPC = H // NCORES            # heads per core = 2
CW = HPC * D                 # per-core width = 128
P = 128
NT = N // P                  # 24 key tiles
KT = IN_F // P               # 8 contraction tiles
SP = 512                     # strip width (queries/nodes)
NSP = N // SP                # 6 strips
SCALE = 1.0 / 8.0            # 1/sqrt(D)


def build_program():
    nc = bacc.Bacc()
    xT = nc.declare_dram_parameter("xT", [IN_F, N], BF16, isOutput=False)
    maskT = nc.declare_dram_parameter("maskT", [N, N], F16, isOutput=False)
    wqT = nc.declare_dram_parameter("wqT", [IN_F, CW], BF16, isOutput=False)
    wkT = nc.declare_dram_parameter("wkT", [IN_F, CW], BF16, isOutput=False)
    wvT = nc.declare_dram_parameter("wvT", [IN_F, CW], BF16, isOutput=False)
    bq = nc.declare_dram_parameter("bq", [1, CW], BF16, isOutput=False)
    woT = nc.declare_dram_parameter("woT", [CW, OUT_F], F32R, isOutput=False)
    ypart = nc.declare_dram_parameter("ypart", [N, OUT_F], BF16, isOutput=True)

    with tile.TileContext(nc) as tc, ExitStack() as ctx:
        cst = ctx.enter_context(tc.tile_pool(name="cst", bufs=1))
        lp = ctx.enter_context(tc.tile_pool(name="lp", bufs=3))       # xs strips
        mtp = ctx.enter_context(tc.tile_pool(name="mtp", bufs=11))    # mask tiles
        ppq = ctx.enter_context(tc.tile_pool(name="ppq", bufs=4))     # p/pm pairs
        ep = ctx.enter_context(tc.tile_pool(name="ep", bufs=4))       # epilogue
        # PSUM: spair 2x2 banks + pso 2 banks + pwork 2x1 banks = 8
        spair = ctx.enter_context(tc.tile_pool(name="spair", bufs=2, space="PSUM"))
        pso = ctx.enter_context(tc.tile_pool(name="pso", bufs=1, space="PSUM"))
        pwork = ctx.enter_context(tc.tile_pool(name="pwork", bufs=2, space="PSUM"))

        ident = cst.tile([P, P], F32)
        make_identity(nc, ident)

        ones_row = cst.tile([1, SP], BF16)
        nc.vector.memset(ones_row[:], 1.0)
        zero_col = cst.tile([P, 1], BF16)
        nc.vector.memset(zero_col[:], 0.0)
        one_col = cst.tile([P, 1], BF16)
        nc.vector.memset(one_col[:], 1.0)

        # persistent activations (bf16)
        q2s = [cst.tile([P, SP], BF16, tag=f"q2s{s}", name=f"q2s{s}")
               for s in range(NSP)]
        k2zs = [[cst.tile([P, SP], BF16, tag=f"k2z{h}_{s}", name=f"k2z{h}_{s}")
                 for s in range(NSP)] for h in range(HPC)]
        vaug = [cst.tile([P, NT, D + 1], BF16, tag=f"vaug{h}", name=f"vaug{h}")
                for h in range(HPC)]
        attn_s = [cst.tile([P, SP], F32R, tag=f"attn{s}", name=f"attn{s}")
                  for s in range(NSP)]
        for h in range(HPC):
            nc.vector.tensor_copy(vaug[h][:, :, D:D + 1],
                                  one_col[:, 0:1, None].to_broadcast([P, NT, 1]))
            osl = slice((1 - h) * D, (2 - h) * D)   # dead half of k2z[h]
            for s in range(NSP):
                nc.vector.tensor_copy(k2zs[h][s][osl, :],
                                      zero_col[osl, 0:1].to_broadcast([D, SP]))

        # PE p-state warmup during the startup DMA window: 24 WAW-chained
        # dummy matmuls into one dedicated PSUM tile (pso ring, bufs=1, so
        # no ring-parity shift for any later allocation)
        warm_w = cst.tile([P, P], BF16)
        nc.vector.memset(warm_w[:], 0.0)
        warm_x = cst.tile([P, SP], BF16)
        nc.vector.memset(warm_x[:], 0.0)
        wps = pso.tile([D + 1, SP], F32, tag="ps_o0", name="warm_ps")
        for _wi in range(24):
            nc.tensor.matmul(wps[:, 0:SP], warm_w[:, 0:D + 1], warm_x[:],
                             start=True, stop=True)

        # weights
        wq_sb = cst.tile([P, KT, CW], BF16)
        for _h2 in range(2):
            nc.sync.dma_start(
                wq_sb[:, 4 * _h2:4 * _h2 + 4, :],
                wqT.rearrange("(k p) m -> p k m", p=P)[:, 4 * _h2:4 * _h2 + 4, :])
        wk_sb = cst.tile([P, KT, CW], BF16)
        for _h2 in range(2):
            nc.sync.dma_start(
                wk_sb[:, 4 * _h2:4 * _h2 + 4, :],
                wkT.rearrange("(k p) m -> p k m", p=P)[:, 4 * _h2:4 * _h2 + 4, :])
        wv_sb = cst.tile([P, KT, CW], BF16)
        for _h2 in range(2):
            nc.sync.dma_start(
                wv_sb[:, 4 * _h2:4 * _h2 + 4, :],
                wvT.rearrange("(k p) m -> p k m", p=P)[:, 4 * _h2:4 * _h2 + 4, :])
        wo_sb = cst.tile([P, OUT_F], F32R)
        bq_sb = cst.tile([1, CW], BF16)
        nc.sync.dma_start(bq_sb[:], bq[:])

        # ---- phase A: k/v projections (needed progressively by all strips);
        #      q-proj is per-strip and deferrable to just before B(s) ----
        def emit_q_strip(s, xs_reuse=None):
            if xs_reuse is None:
                xq_c = [lp.tile([P, 2, SP], BF16, tag=f"xq_c{c}", name=f"xq_c{c}")
                        for c in range(KT // 2)]
                for c in range(KT // 2):
                    nc.sync.dma_start(
                        xq_c[c][:],
                        xT.rearrange("(k p) n -> p k n", p=P)[:, 2 * c:2 * c + 2,
                                                              s * SP:(s + 1) * SP])
            else:
                xq_c = xs_reuse
            ps = pwork.tile([P, SP], F32, tag="pw", name="ps_q")
            for k in range(KT):
                nc.tensor.matmul(ps[:], wq_sb[:, k, :], xq_c[k // 2][:, k % 2, :],
                                 start=(k == 0), stop=False)
            nc.tensor.matmul(ps[:], bq_sb[:], ones_row[:],
                             start=False, stop=True)
            nc.vector.tensor_copy(q2s[s][:], ps[:])

        def emit_kv_strip(s):
            xs_c = [lp.tile([P, 2, SP], BF16, tag=f"xs_c{c}", name=f"xs_c{c}")
                    for c in range(KT // 2)]
            for c in range(KT // 2):
                if s == 0:
                    for j in range(2):
                        nc.sync.dma_start(
                            xs_c[c][:, j, :],
                            xT.rearrange("(k p) n -> p k n", p=P)[:, 2 * c + j,
                                                                  0:SP])
                else:
                    nc.sync.dma_start(
                        xs_c[c][:],
                        xT.rearrange("(k p) n -> p k n", p=P)[:, 2 * c:2 * c + 2,
                                                              s * SP:(s + 1) * SP])

            def xsk(k):
                return xs_c[k // 2][:, k % 2, :]

            # k-proj (no bias)
            ps = pwork.tile([P, SP], F32, tag="pw", name="ps_k")
            for k in range(KT):
                nc.tensor.matmul(ps[:], wk_sb[:, k, :], xsk(k),
                                 start=(k == 0), stop=(k == KT - 1))
            for h in range(HPC):
                hsl = slice(h * D, (h + 1) * D)
                nc.vector.tensor_copy(k2zs[h][s][hsl, :], ps[hsl, :])
            # v-proj (no bias), then transpose into vaug
            ps = pwork.tile([P, SP], F32, tag="pw", name="ps_v")
            for k in range(KT):
                nc.tensor.matmul(ps[:], wv_sb[:, k, :], xsk(k),
                                 start=(k == 0), stop=(k == KT - 1))
            v2Ts = lp.tile([P, SP], F32, tag="v2Ts", name="v2Ts")
            nc.vector.tensor_copy(v2Ts[:], ps[:])
            for b in range(SP // P):
                t = s * (SP // P) + b
                ps_t = pwork.tile([P, SP], F32, tag="pw", name="ps_t")
                nc.tensor.transpose(ps_t[:, 0:P], v2Ts[:, b * P:(b + 1) * P],
                                    ident[:])
                for h in range(HPC):
                    nc.vector.tensor_copy(vaug[h][:, t, 0:D],
                                          ps_t[:, h * D:h * D + D])
            return xs_c

        # ---- phase B: one (strip, key-tile) step, PV deferred via pend ----
        def emit_pv(ps_o_t, mk, pm):
            for h in range(HPC):
                nc.tensor.matmul(ps_o_t[h][:], vaug[h][:, mk, :], pm[:, h, :],
                                 start=(mk == 0), stop=(mk == NT - 1))

        def emit_B(s, mk, pend):
            mt = mtp.tile([P, SP], F16, tag="mt", name="mt")
            nc.sync.dma_start(mt[:], maskT[mk * P:(mk + 1) * P,
                                           s * SP:(s + 1) * SP])
            sp_ = spair.tile([P, HPC, SP], F32, tag="sp", name="sp")
            for h in range(HPC):
                nc.tensor.matmul(sp_[:, h, :],
                                 k2zs[h][mk // 4][:, (mk % 4) * P:(mk % 4 + 1) * P],
                                 q2s[s][:], start=True, stop=True)
            p_ = ppq.tile([P, HPC, SP], BF16, tag="p", name="p")
            nc.scalar.activation(p_[:], sp_[:], AF.Exp, scale=SCALE)
            pm = ppq.tile([P, HPC, SP], BF16, tag="pm", name="pm")
            nc.vector.tensor_tensor(pm[:], p_[:],
                                    mt[:, None, :].to_broadcast([P, HPC, SP]),
                                    ALU.mult)
            pend.append((ps_o, mk, pm))
            if len(pend) > 2:
                emit_pv(*pend.pop(0))

        # ---- epilogue, split: head frees PSUM early; tail does out_proj ----
        def emit_ep_head(s, ps_o_s):
            osbs = []
            for h in range(HPC):
                osb = ep.tile([D + 1, SP], F32, tag=f"osb{h}", name=f"osb{h}")
                nc.vector.tensor_copy(osb[:], ps_o_s[h][:])
                osbs.append(osb)
            parts = []
            for h in range(HPC):
                osb = osbs[h]
                zrow = ep.tile([1, SP], F32, tag="zrow", name=f"zrow{h}")
                nc.vector.tensor_copy(zrow[:], osb[D:D + 1, :])
                recip = ep.tile([1, SP], F32, tag="recip", name=f"recip{h}")
                nc.vector.reciprocal_approx_fast(recip[:], zrow[:])
                bc = ep.tile([D, SP], F32, tag=f"bc{h}", name=f"bc{h}")
                nc.gpsimd.partition_broadcast(bc[:], recip[:])
                parts.append((osb, bc))
            return parts

        def emit_ep_tail(s, parts, last=False):
            for h in range(HPC):
                osb, bc = parts[h]
                nc.vector.tensor_tensor(attn_s[s][h * D:(h + 1) * D, :],
                                        osb[0:D, :], bc[:], ALU.mult)
            for b in range(SP // P):
                t = s * (SP // P) + b
                ys = ep.tile([P, OUT_F], BF16, tag="ys", name="ys")
                for f in range(OUT_F // SP):
                    ps_y = pwork.tile([P, SP], F32, tag="pw", name="ps_y")
                    nc.tensor.matmul(ps_y[:],
                                     attn_s[s][:, b * P:(b + 1) * P],
                                     wo_sb[:, f * SP:(f + 1) * SP],
                                     start=True, stop=True)
                    nc.vector.tensor_copy(ys[:, f * SP:(f + 1) * SP],
                                          ps_y[:])
                    nc.sync.dma_start(
                        ypart[t * P:(t + 1) * P, f * SP:(f + 1) * SP],
                        ys[:, f * SP:(f + 1) * SP])

        # ---- interleaved emission ----
        xs0 = emit_kv_strip(0)
        emit_q_strip(0, xs_reuse=xs0)
        emit_kv_strip(1)
        ps_o = [pso.tile([D + 1, SP], F32, tag=f"ps_o{h}", name=f"ps_o{h}_0")
                for h in range(HPC)]
        pend = []
        for mk in range(0, 8):
            emit_B(0, mk, pend)
        nc.sync.dma_start(wo_sb[:], woT[:])
        emit_kv_strip(2)
        for mk in range(8, 12):
            emit_B(0, mk, pend)
        emit_kv_strip(3)
        emit_q_strip(1)
        for mk in range(12, 16):
            emit_B(0, mk, pend)
        emit_kv_strip(4)
        for mk in range(16, 20):
            emit_B(0, mk, pend)
        emit_kv_strip(5)
        for mk in range(20, 24):
            emit_B(0, mk, pend)
        for s in range(1, NSP):
            ps_o_prev, pend_prev = ps_o, pend
            ps_o = [pso.tile([D + 1, SP], F32, tag=f"ps_o{h}", name=f"ps_o{h}_{s}")
                    for h in range(HPC)]
            pend = []
            for mk in range(0, 4):
                emit_B(s, mk, pend)
            for args in pend_prev:
                emit_pv(*args)
            parts = emit_ep_head(s - 1, ps_o_prev)
            emit_ep_tail(s - 1, parts)
            for mk in range(4, 8):
                emit_B(s, mk, pend)
            if s + 1 < NSP:
                emit_q_strip(s + 1)
            for mk in range(8, NT):
                emit_B(s, mk, pend)
        for args in pend:
            emit_pv(*args)
        parts = emit_ep_head(NSP - 1, ps_o)
        emit_ep_tail(NSP - 1, parts, last=True)

    nc.compile()
    return nc


_PROGRAM = None
LAST_RESULTS = None


def _get_program():
    global _PROGRAM
    if _PROGRAM is None:
        _PROGRAM = build_program()
    return _PROGRAM


def _softplus(x):
    x = np.asarray(x, np.float32)
    return np.logaddexp(0.0, x).astype(np.float32)


def host_prep(inputs):
    x = np.asarray(inputs["x"], np.float32)
    edge_index = np.asarray(inputs["edge_index"])
    edge_type = np.asarray(inputs["edge_type"])
    etw = np.asarray(inputs["edge_type_weights"], np.float32)

    def f32(k):
        return np.asarray(inputs[k], np.float32)

    # compose the two linear layers: q2 = x @ (wiq@wq).T + (wiq@bq + biq)
    WQ = f32("wiq") @ f32("wq")
    bQ = f32("wiq") @ f32("bq") + f32("biq")
    WK = f32("wik") @ f32("wk")
    WV = f32("wiv") @ f32("wv")
    bV = f32("wiv") @ f32("bv") + f32("biv")
    wo = f32("wo")
    bo = f32("bo")
    # bk cancels in softmax; bv contributes exactly bV @ wo.T (attn rows sum
    # to 1), folded into the host-side output bias.
    y_base = (bV @ wo.T + bo).astype(np.float32)

    # multiplicative mask, transposed: maskT[m, n] = exp(add_mask[n, m])
    w = _softplus(etw)
    M = np.zeros((N, N), np.float32)
    src, dst = edge_index[0], edge_index[1]
    ew = np.exp(w).astype(np.float32)
    M[src, dst] = ew[edge_type - 1]            # last write wins, like jax .at[].set
    diag = np.diagonal(M).copy()
    didx = np.arange(N)
    M[didx, didx] = np.where(diag == 0.0, ew[3], diag)
    maskT = np.ascontiguousarray(M.T).astype(np.float16)

    xT = np.ascontiguousarray(x.T).astype(ml_dtypes.bfloat16)

    bf = ml_dtypes.bfloat16
    in_maps = []
    for c in range(NCORES):
        rs = slice(c * CW, (c + 1) * CW)
        in_maps.append({
            "xT": xT,
            "maskT": maskT,
            "wqT": np.ascontiguousarray(WQ[rs].T).astype(bf),
            "wkT": np.ascontiguousarray(WK[rs].T).astype(bf),
            "wvT": np.ascontiguousarray(WV[rs].T).astype(bf),
            "bq": np.ascontiguousarray(bQ[rs][None, :]).astype(bf),
            "woT": np.ascontiguousarray(wo[:, rs].T),
        })
    return in_maps, y_base


def kernel(**inputs) -> np.ndarray:
    global LAST_RESULTS
    in_maps, y_base = host_prep(inputs)
    nc = _get_program()
    trace = bool(os.environ.get("KERNEL_TRACE"))
    res = run_bass_kernel_spmd(nc, in_maps, list(range(NCORES)), trace=trace)
    LAST_RESULTS = res
    y = y_base[None, :].astype(np.float32).repeat(N, axis=0)
    for c in range(NCORES):
        y += res.results[c]["ypart"].astype(np.float32)
    return y
